# revision 34
# baseline (speedup 1.0000x reference)
"""Trainium2 kernel for nn_DenseGeneral fp8-qdq forward.

Reference computes: out = qdq_e4m3(inputs) @ qdq_e4m3(kernel) + bf16_round(bias)
(forward pass of fp8-aware DenseGeneral; scale/amax updates only live in the
custom_vjp residuals and do not affect the forward output).

Strategy:
- Host: quantize inputs/kernel to e4m3 exactly as the reference does (scales are
  ones in this problem, but general scales are folded back into the output).
  OCP e4m3fn bit patterns == TRN FP8_EXP4 for |v| <= 240, which holds here.
- 4x2 grid: shard rows of inp_mat 4-way and kernel columns 2-way; per core the
  resident inputs are xw 4 MB + wn 8 MB fp8 (12 MB SBUF), 1024 DoubleRow MMs.
- Device: all-SBUF-resident fp8 DoubleRow matmuls at the 1 col/cycle floor
  (512 cycles + ~3ns NX per MM, measured 216 ns/MM at 2.4 GHz; SwInterleave
  benches identical, so this IS the fp8 peak). All inputs stream on the sync
  HWDGE queue in c-major consumption order, with the first group's wn/xw
  split per k-pair so its j-ascending MMs unblock as each chunk lands; the
  DRAM images are chunk-major (byte-identical to the SBUF resident tiles),
  so every input DMA is a fully contiguous copy and the ramp runs ~1.5us
  faster than the strided layout.
  8 PSUM bufs (full PSUM) so copyback latency never stalls the PE; outputs
  are downcast to bf16 on the PSUM->SBUF copy, halving output DMA; the last
  group is split 384+128 cols to shorten the post-stream drain chain.
  24 dummy FD-256 DR MMs (~5.2us busy) open the PE HAM clock gate while the
  DMA prefix lands (shorter warmups leave idle holes that re-close the gate
  at half clock, which costs far more than the warmup overrun).
- Host: gather bf16, upcast, apply scale product + bias, reshape.
"""

import numpy as np
import ml_dtypes

P = 128
B, S, D, F = 4, 2048, 2048, 8192
M = B * S  # 8192 rows of inp_mat
GRID_M, GRID_N = 4, 2
M_LOC = M // GRID_M  # 2048
N_LOC = F // GRID_N  # 4096
N_CORES = 8
N_WARM = 24  # leading dummy FD-256 DR MMs (HAM clock-gate opener, ~5.2us busy)

_PROGRAM = None  # (nc, kxm_name, kxn_name, mxn_name)


def _build_program():
    """Resident-input fp8 DoubleRow matmul: per core
    out[M_LOC, N_LOC] = kxm.T @ kxn, inputs SBUF-resident (12 MB),
    1024 DoubleRow MMs streaming at the 512-cycle floor, bf16 output."""
    global _PROGRAM
    if _PROGRAM is not None:
        return _PROGRAM
    import concourse.bacc as bacc
    import concourse.mybir as mybir
    import concourse.tile as tile

    KO = D // P  # 16 k subtiles
    KP = KO // 2  # 8 k pairs
    MT = M_LOC // P  # 16 m tiles
    NCH = N_LOC // 512  # 8 n chunks

    nc = bacc.Bacc("TRN2", target_bir_lowering=False, debug=False)
    dt = mybir.dt
    DR = mybir.MatmulPerfMode.DoubleRow

    # Warm tile allocated and memset BEFORE TileContext: the memset runs in
    # the pre-barrier prologue window next to the framework const memsets,
    # and the entry barrier orders it before any tensor work — so the first
    # warm LDWEIGHTS is not gated by a post-barrier memset (~1us earlier).
    wz_t = nc.alloc_sbuf_tensor("warm_z0", [P, 2, 256], dt.float8e4)
    nc.gpsimd.memset(wz_t.ap(), 0)
    nc.all_engine_barrier()
    wz = wz_t.ap()

    with tile.TileContext(nc) as tc:
        # Chunk-major DRAM images: the SBUF-resident layout and the DRAM
        # layout are byte-identical, so every input DMA is a fully
        # contiguous copy (no small strided runs throttling the ramp).
        kxm = nc.dram_tensor(
            "kxm", (P, MT, KO, 128), dt.float8e4, kind="ExternalInput"
        ).ap()
        kxn = nc.dram_tensor(
            "kxn", (P, NCH, KO, 512), dt.float8e4, kind="ExternalInput"
        ).ap()
        mxn = nc.dram_tensor(
            "mxn", (P, MT, N_LOC), dt.bfloat16, kind="ExternalOutput"
        ).ap()

        with (
            tc.tile_pool(name="resident", bufs=1) as res_pool,
            tc.tile_pool(name="psum", bufs=8, space="PSUM") as psum_pool,
            tc.tile_pool(name="outp", bufs=16) as out_pool,
        ):
            wps = psum_pool.tile([P, 512], dt.float32, name="ps")

            def warm_mm():
                nc.tensor.matmul(
                    wps[:, 0:256],
                    wz[:, :, 0:128],
                    wz[:],
                    start=True,
                    stop=True,
                    perf_mode=DR,
                    skip_group_check=True,
                )

            for _ in range(N_WARM):
                warm_mm()

            # Chunk-major resident tiles (t-major xw, c-major wn): each DMA
            # source range and destination range are contiguous bytes.
            xw_all = res_pool.tile([P, MT, KP, 2, 128], dt.float8e4, name="xw_all")
            wn_all = res_pool.tile([P, NCH, KP, 2, 512], dt.float8e4, name="wn_all")

            def ld_wn(jlo, jhi, c):
                nc.sync.dma_start(
                    wn_all[:, c, jlo:jhi], kxn[:, c, 2 * jlo : 2 * jhi]
                )

            def ld_xw(jlo, jhi, t):
                nc.sync.dma_start(
                    xw_all[:, t, jlo:jhi], kxm[:, t, 2 * jlo : 2 * jhi]
                )

            def ld_wn_c(clo, chi):
                nc.sync.dma_start(wn_all[:, clo:chi], kxn[:, clo:chi])

            def ld_xw_t(tlo, thi):
                nc.sync.dma_start(xw_all[:, tlo:thi], kxm[:, tlo:thi])

            # Single sync FIFO ring, ordered to match c-major consumption:
            # group (c0,t0) unblocks pair by pair, then xw tiles land just
            # ahead of their t-groups, wn chunks well ahead of their c-phase.
            ld_wn(0, 1, 0)
            ld_xw(0, 2, 0)
            ld_wn(1, 2, 0)
            ld_xw(2, 8, 0)
            ld_wn(2, 3, 0)
            ld_wn(3, 4, 0)
            ld_wn(4, 6, 0)
            ld_wn(6, 8, 0)
            ld_xw_t(1, 2)
            ld_xw_t(2, 3)
            ld_xw_t(3, 4)
            ld_xw_t(4, 6)
            ld_xw_t(6, 8)
            ld_xw_t(8, 12)
            ld_xw_t(12, MT)
            ld_wn(0, 4, 1)
            ld_wn(4, 8, 1)
            ld_wn_c(2, 4)
            ld_wn_c(4, 6)
            ld_wn_c(6, NCH)

            def group(c, t):
                ps = psum_pool.tile([P, 512], dt.float32, name="ps")
                for j in range(KP):
                    nc.tensor.matmul(
                        ps[:],
                        xw_all[:, t, j],
                        wn_all[:, c, j],
                        start=(j == 0),
                        stop=(j == KP - 1),
                        perf_mode=DR,
                    )
                oc = out_pool.tile([P, 512], dt.bfloat16, name="oc")
                nc.vector.tensor_copy(oc[:], ps[:])
                nc.scalar.dma_start(mxn[:, t, c * 512 : (c + 1) * 512], oc[:])

            def group_split(c, t):
                """Last group: two 256-col halves so the final PSUM->SBUF
                copy + output DMA chain after the last matmul is half-size
                (the first half's copy/DMA overlap the second half's MMs)."""
                for lo, w in ((0, 384), (384, 128)):
                    ps = psum_pool.tile([P, w], dt.float32, name="ps")
                    for j in range(KP):
                        nc.tensor.matmul(
                            ps[:],
                            xw_all[:, t, j],
                            wn_all[:, c, j, :, lo : lo + w],
                            start=(j == 0),
                            stop=(j == KP - 1),
                            perf_mode=DR,
                        )
                    oc = out_pool.tile([P, w], dt.bfloat16, name="oc")
                    nc.vector.tensor_copy(oc[:], ps[:])
                    # Final 128-col half goes out on the (idle) sync queue so
                    # its issue is not serialized behind the 384-col half's.
                    eng = nc.sync if lo else nc.scalar
                    eng.dma_start(
                        mxn[:, t, c * 512 + lo : c * 512 + lo + w], oc[:]
                    )

            # Main stream: c-major.
            for c in range(NCH):
                for t in range(MT):
                    if c == NCH - 1 and t == MT - 1:
                        group_split(c, t)
                    else:
                        group(c, t)

    nc.compile()
    _PROGRAM = (nc, "kxm", "kxn", "mxn")
    return _PROGRAM


def _qdq_e4m3(x, scale):
    """fp32 -> e4m3 with the reference's scale/clip semantics; TRN-fp8 view."""
    if scale != 1.0:
        x = x / np.float32(scale)
    q = np.clip(x, -448.0, 448.0).astype(ml_dtypes.float8_e4m3fn)
    return q.view(ml_dtypes.float8_e4m3)


def _make_shards(xq, wq):
    """Per-core chunk-major DRAM input images for the 4x2 grid (shared where
    slices coincide). kxm [P, MT, KO, 128] with [p,t,ko,m] = x[t*128+m, ko*128+p];
    kxn [P, NCH, KO, 512] with [p,c,ko,n] = w[ko*128+p, c*512+n] — byte layouts
    identical to the SBUF resident tiles, so input DMAs are contiguous."""
    KO, MT, NCH = D // P, M_LOC // P, N_LOC // 512
    kxm_shards = []
    for mi in range(GRID_M):
        xs = xq[mi * M_LOC : (mi + 1) * M_LOC, :]  # [M_LOC, D]
        kxm_shards.append(
            np.ascontiguousarray(
                xs.reshape(MT, P, KO, P).transpose(3, 0, 2, 1)
            )
        )
    kxn_shards = []
    for ni in range(GRID_N):
        ws = wq[:, ni * N_LOC : (ni + 1) * N_LOC]  # [D, N_LOC]
        kxn_shards.append(
            np.ascontiguousarray(
                ws.reshape(KO, P, NCH, 512).transpose(1, 2, 0, 3)
            )
        )
    return kxm_shards, kxn_shards


def _ensure_axon_hooks_stub():
    """The trimmed image lacks antenv.axon_hooks; if BASS_TRACE is set in the
    environment, run_bass_kernel_spmd would crash importing it. Provide a
    no-op hook module (bass_utils degrades gracefully on a None hook)."""
    import sys
    import types

    try:
        import antenv.axon_hooks  # noqa: F401
    except ImportError:
        mod = types.ModuleType("antenv.axon_hooks")
        mod.get_axon_ntff_profile_hook = lambda: None
        mod.set_axon_ntff_profile_hook = lambda hook: None
        sys.modules["antenv.axon_hooks"] = mod


def kernel(
    inputs,
    kernel,
    bias,
    input_scale,
    kernel_scale,
    output_grad_scale,
    input_amax_history,
    kernel_amax_history,
    output_grad_amax_history,
):
    _ensure_axon_hooks_stub()
    from concourse.bass_utils import run_bass_kernel_spmd

    nc, kxm_name, kxn_name, mxn_name = _build_program()

    x = np.asarray(inputs, dtype=np.float32).reshape(M, D)
    w = np.asarray(kernel, dtype=np.float32)
    s_in = float(np.asarray(input_scale).reshape(-1)[0])
    s_k = float(np.asarray(kernel_scale).reshape(-1)[0])

    xq = _qdq_e4m3(x, s_in)  # [M, D] fp8
    wq = _qdq_e4m3(w, s_k)  # [D, F] fp8

    kxm_shards, kxn_shards = _make_shards(xq, wq)

    in_maps = []
    for c in range(N_CORES):
        mi, ni = divmod(c, GRID_N)
        in_maps.append({kxm_name: kxm_shards[mi], kxn_name: kxn_shards[ni]})

    res = run_bass_kernel_spmd(nc, in_maps, core_ids=list(range(N_CORES)))

    out = np.empty((M, F), dtype=np.float32)
    for c in range(N_CORES):
        mi, ni = divmod(c, GRID_N)
        block = res.results[c][mxn_name]  # [P, M_LOC//P, N_LOC] bf16
        out[mi * M_LOC : (mi + 1) * M_LOC, ni * N_LOC : (ni + 1) * N_LOC] = (
            np.asarray(block)
            .view(ml_dtypes.bfloat16)
            .astype(np.float32)
            .transpose(1, 0, 2)
            .reshape(M_LOC, N_LOC)
        )

    sprod = s_in * s_k
    if sprod != 1.0:
        out *= np.float32(sprod)

    b = np.asarray(bias, dtype=np.float32)
    b = b.astype(ml_dtypes.bfloat16).astype(np.float32)
    if np.any(b):
        out += b[None, :]

    return out.reshape(B, S, F)


# revision 38
# speedup vs baseline: 1.0164x; 1.0164x over previous
"""Trainium2 kernel for nn_DenseGeneral fp8-qdq forward.

Reference computes: out = qdq_e4m3(inputs) @ qdq_e4m3(kernel) + bf16_round(bias)
(forward pass of fp8-aware DenseGeneral; scale/amax updates only live in the
custom_vjp residuals and do not affect the forward output).

Strategy:
- Host: quantize inputs/kernel to e4m3 exactly as the reference does (scales are
  ones in this problem, but general scales are folded back into the output).
  OCP e4m3fn bit patterns == TRN FP8_EXP4 for |v| <= 240, which holds here.
- 4x2 grid: shard rows of inp_mat 4-way and kernel columns 2-way; per core the
  resident inputs are xw 4 MB + wn 8 MB fp8 (12 MB SBUF), 1024 DoubleRow MMs.
- Device: all-SBUF-resident fp8 DoubleRow matmuls at the 1 col/cycle floor
  (512 cycles + ~3ns NX per MM, measured 216 ns/MM at 2.4 GHz; SwInterleave
  benches identical, so this IS the fp8 peak). All inputs stream on the sync
  HWDGE queue in c-major consumption order, with the first group's wn/xw
  split per k-pair so its j-ascending MMs unblock as each chunk lands; the
  DRAM images are chunk-major (byte-identical to the SBUF resident tiles),
  so every input DMA is a fully contiguous copy and the ramp runs ~1.5us
  faster than the strided layout.
  8 PSUM bufs (full PSUM) so copyback latency never stalls the PE; outputs
  are downcast to bf16 on the PSUM->SBUF copy, halving output DMA; the last
  group is split 384+128 cols to shorten the post-stream drain chain.
  24 dummy FD-256 DR MMs (~5.2us busy) open the PE HAM clock gate while the
  DMA prefix lands (shorter warmups leave idle holes that re-close the gate
  at half clock, which costs far more than the warmup overrun).
- Host: gather bf16, upcast, apply scale product + bias, reshape.
"""

import numpy as np
import ml_dtypes

P = 128
B, S, D, F = 4, 2048, 2048, 8192
M = B * S  # 8192 rows of inp_mat
GRID_M, GRID_N = 4, 2
M_LOC = M // GRID_M  # 2048
N_LOC = F // GRID_N  # 4096
N_CORES = 8
N_WARM = 24  # leading dummy FD-256 DR MMs (HAM clock-gate opener, ~5.2us busy)

_PROGRAM = None  # (nc, kxm_name, kxn_name, mxn_name)


def _build_program():
    """Resident-input fp8 DoubleRow matmul: per core
    out[M_LOC, N_LOC] = kxm.T @ kxn, inputs SBUF-resident (12 MB),
    1024 DoubleRow MMs streaming at the 512-cycle floor, bf16 output."""
    global _PROGRAM
    if _PROGRAM is not None:
        return _PROGRAM
    import concourse.bacc as bacc
    import concourse.mybir as mybir
    import concourse.tile as tile

    KO = D // P  # 16 k subtiles
    KP = KO // 2  # 8 k pairs
    MT = M_LOC // P  # 16 m tiles
    NCH = N_LOC // 512  # 8 n chunks

    nc = bacc.Bacc("TRN2", target_bir_lowering=False, debug=False)
    dt = mybir.dt
    DR = mybir.MatmulPerfMode.DoubleRow

    with tile.TileContext(nc) as tc:
        # Chunk-major DRAM images: the SBUF-resident layout and the DRAM
        # layout are byte-identical, so every input DMA is a fully
        # contiguous copy (no small strided runs throttling the ramp).
        kxm = nc.dram_tensor(
            "kxm", (P, MT, KO, 128), dt.float8e4, kind="ExternalInput"
        ).ap()
        kxn = nc.dram_tensor(
            "kxn", (P, NCH, KO, 512), dt.float8e4, kind="ExternalInput"
        ).ap()
        mxn = nc.dram_tensor(
            "mxn", (P, MT, N_LOC), dt.bfloat16, kind="ExternalOutput"
        ).ap()

        with (
            tc.tile_pool(name="resident", bufs=1) as res_pool,
            tc.tile_pool(name="psum", bufs=8, space="PSUM") as psum_pool,
            tc.tile_pool(name="outp", bufs=16) as out_pool,
            tc.tile_pool(name="warm", bufs=1) as warm_pool,
        ):
            # Warm tile: stationary and moving operands of the dummy MMs both
            # read from it; one small vector memset initializes it.
            wz = warm_pool.tile([P, 2, 256], dt.float8e4, name="warm_z")
            nc.vector.memset(wz[:], 0.0)
            wps = psum_pool.tile([P, 512], dt.float32, name="ps")

            def warm_mm():
                nc.tensor.matmul(
                    wps[:, 0:256],
                    wz[:, :, 0:128],
                    wz[:],
                    start=True,
                    stop=True,
                    perf_mode=DR,
                    skip_group_check=True,
                )

            for _ in range(N_WARM):
                warm_mm()

            # Chunk-major resident tiles (t-major xw, c-major wn): each DMA
            # source range and destination range are contiguous bytes.
            xw_all = res_pool.tile([P, MT, KP, 2, 128], dt.float8e4, name="xw_all")
            wn_all = res_pool.tile([P, NCH, KP, 2, 512], dt.float8e4, name="wn_all")

            def ld_wn(jlo, jhi, c):
                nc.sync.dma_start(
                    wn_all[:, c, jlo:jhi], kxn[:, c, 2 * jlo : 2 * jhi]
                )

            def ld_xw(jlo, jhi, t):
                nc.sync.dma_start(
                    xw_all[:, t, jlo:jhi], kxm[:, t, 2 * jlo : 2 * jhi]
                )

            def ld_wn_c(clo, chi):
                nc.sync.dma_start(wn_all[:, clo:chi], kxn[:, clo:chi])

            def ld_xw_t(tlo, thi):
                nc.sync.dma_start(xw_all[:, tlo:thi], kxm[:, tlo:thi])

            # Single sync FIFO ring, ordered to match c-major consumption:
            # group (c0,t0) unblocks pair by pair, then xw tiles land just
            # ahead of their t-groups, wn chunks well ahead of their c-phase.
            ld_wn(0, 1, 0)
            ld_xw(0, 2, 0)
            ld_wn(1, 2, 0)
            ld_xw(2, 8, 0)
            ld_wn(2, 3, 0)
            ld_wn(3, 4, 0)
            ld_wn(4, 6, 0)
            ld_wn(6, 8, 0)
            ld_xw_t(1, 2)
            ld_xw_t(2, 3)
            ld_xw_t(3, 4)
            ld_xw_t(4, 6)
            ld_xw_t(6, 8)
            ld_xw_t(8, 12)
            ld_xw_t(12, MT)
            ld_wn(0, 4, 1)
            ld_wn(4, 8, 1)
            ld_wn_c(2, 4)
            ld_wn_c(4, 6)
            ld_wn_c(6, NCH)

            def group(c, t):
                ps = psum_pool.tile([P, 512], dt.float32, name="ps")
                for j in range(KP):
                    nc.tensor.matmul(
                        ps[:],
                        xw_all[:, t, j],
                        wn_all[:, c, j],
                        start=(j == 0),
                        stop=(j == KP - 1),
                        perf_mode=DR,
                    )
                oc = out_pool.tile([P, 512], dt.bfloat16, name="oc")
                nc.vector.tensor_copy(oc[:], ps[:])
                nc.scalar.dma_start(mxn[:, t, c * 512 : (c + 1) * 512], oc[:])

            def group_split(c, t):
                """Last group: two 256-col halves so the final PSUM->SBUF
                copy + output DMA chain after the last matmul is half-size
                (the first half's copy/DMA overlap the second half's MMs)."""
                for lo, w in ((0, 384), (384, 128)):
                    ps = psum_pool.tile([P, w], dt.float32, name="ps")
                    for j in range(KP):
                        nc.tensor.matmul(
                            ps[:],
                            xw_all[:, t, j],
                            wn_all[:, c, j, :, lo : lo + w],
                            start=(j == 0),
                            stop=(j == KP - 1),
                            perf_mode=DR,
                        )
                    oc = out_pool.tile([P, w], dt.bfloat16, name="oc")
                    nc.vector.tensor_copy(oc[:], ps[:])
                    nc.scalar.dma_start(
                        mxn[:, t, c * 512 + lo : c * 512 + lo + w], oc[:]
                    )

            # Main stream: c-major.
            for c in range(NCH):
                for t in range(MT):
                    if c == NCH - 1 and t == MT - 1:
                        group_split(c, t)
                    else:
                        group(c, t)

    nc.compile()
    _PROGRAM = (nc, "kxm", "kxn", "mxn")
    return _PROGRAM


def _qdq_e4m3(x, scale):
    """fp32 -> e4m3 with the reference's scale/clip semantics; TRN-fp8 view."""
    if scale != 1.0:
        x = x / np.float32(scale)
    q = np.clip(x, -448.0, 448.0).astype(ml_dtypes.float8_e4m3fn)
    return q.view(ml_dtypes.float8_e4m3)


def _make_shards(xq, wq):
    """Per-core chunk-major DRAM input images for the 4x2 grid (shared where
    slices coincide). kxm [P, MT, KO, 128] with [p,t,ko,m] = x[t*128+m, ko*128+p];
    kxn [P, NCH, KO, 512] with [p,c,ko,n] = w[ko*128+p, c*512+n] — byte layouts
    identical to the SBUF resident tiles, so input DMAs are contiguous."""
    KO, MT, NCH = D // P, M_LOC // P, N_LOC // 512
    kxm_shards = []
    for mi in range(GRID_M):
        xs = xq[mi * M_LOC : (mi + 1) * M_LOC, :]  # [M_LOC, D]
        kxm_shards.append(
            np.ascontiguousarray(
                xs.reshape(MT, P, KO, P).transpose(3, 0, 2, 1)
            )
        )
    kxn_shards = []
    for ni in range(GRID_N):
        ws = wq[:, ni * N_LOC : (ni + 1) * N_LOC]  # [D, N_LOC]
        kxn_shards.append(
            np.ascontiguousarray(
                ws.reshape(KO, P, NCH, 512).transpose(1, 2, 0, 3)
            )
        )
    return kxm_shards, kxn_shards


def _ensure_axon_hooks_stub():
    """The trimmed image lacks antenv.axon_hooks; if BASS_TRACE is set in the
    environment, run_bass_kernel_spmd would crash importing it. Provide a
    no-op hook module (bass_utils degrades gracefully on a None hook)."""
    import sys
    import types

    try:
        import antenv.axon_hooks  # noqa: F401
    except ImportError:
        mod = types.ModuleType("antenv.axon_hooks")
        mod.get_axon_ntff_profile_hook = lambda: None
        mod.set_axon_ntff_profile_hook = lambda hook: None
        sys.modules["antenv.axon_hooks"] = mod


def kernel(
    inputs,
    kernel,
    bias,
    input_scale,
    kernel_scale,
    output_grad_scale,
    input_amax_history,
    kernel_amax_history,
    output_grad_amax_history,
):
    _ensure_axon_hooks_stub()
    from concourse.bass_utils import run_bass_kernel_spmd

    nc, kxm_name, kxn_name, mxn_name = _build_program()

    x = np.asarray(inputs, dtype=np.float32).reshape(M, D)
    w = np.asarray(kernel, dtype=np.float32)
    s_in = float(np.asarray(input_scale).reshape(-1)[0])
    s_k = float(np.asarray(kernel_scale).reshape(-1)[0])

    xq = _qdq_e4m3(x, s_in)  # [M, D] fp8
    wq = _qdq_e4m3(w, s_k)  # [D, F] fp8

    kxm_shards, kxn_shards = _make_shards(xq, wq)

    in_maps = []
    for c in range(N_CORES):
        mi, ni = divmod(c, GRID_N)
        in_maps.append({kxm_name: kxm_shards[mi], kxn_name: kxn_shards[ni]})

    res = run_bass_kernel_spmd(nc, in_maps, core_ids=list(range(N_CORES)))

    out = np.empty((M, F), dtype=np.float32)
    for c in range(N_CORES):
        mi, ni = divmod(c, GRID_N)
        block = res.results[c][mxn_name]  # [P, M_LOC//P, N_LOC] bf16
        out[mi * M_LOC : (mi + 1) * M_LOC, ni * N_LOC : (ni + 1) * N_LOC] = (
            np.asarray(block)
            .view(ml_dtypes.bfloat16)
            .astype(np.float32)
            .transpose(1, 0, 2)
            .reshape(M_LOC, N_LOC)
        )

    sprod = s_in * s_k
    if sprod != 1.0:
        out *= np.float32(sprod)

    b = np.asarray(bias, dtype=np.float32)
    b = b.astype(ml_dtypes.bfloat16).astype(np.float32)
    if np.any(b):
        out += b[None, :]

    return out.reshape(B, S, F)


# revision 40
# speedup vs baseline: 1.0173x; 1.0009x over previous
"""Trainium2 kernel for nn_DenseGeneral fp8-qdq forward.

Reference computes: out = qdq_e4m3(inputs) @ qdq_e4m3(kernel) + bf16_round(bias)
(forward pass of fp8-aware DenseGeneral; scale/amax updates only live in the
custom_vjp residuals and do not affect the forward output).

Strategy:
- Host: quantize inputs/kernel to e4m3 exactly as the reference does (scales are
  ones in this problem, but general scales are folded back into the output).
  OCP e4m3fn bit patterns == TRN FP8_EXP4 for |v| <= 240, which holds here.
- 4x2 grid: shard rows of inp_mat 4-way and kernel columns 2-way; per core the
  resident inputs are xw 4 MB + wn 8 MB fp8 (12 MB SBUF), 1024 DoubleRow MMs.
- Device: all-SBUF-resident fp8 DoubleRow matmuls at the 1 col/cycle floor
  (512 cycles + ~3ns NX per MM, measured 216 ns/MM at 2.4 GHz; SwInterleave
  benches identical, so this IS the fp8 peak). All inputs stream on the sync
  HWDGE queue in c-major consumption order, with the first group's wn/xw
  split per k-pair so its j-ascending MMs unblock as each chunk lands; the
  DRAM images are chunk-major (byte-identical to the SBUF resident tiles),
  so every input DMA is a fully contiguous copy and the ramp runs ~1.5us
  faster than the strided layout.
  8 PSUM bufs (full PSUM) so copyback latency never stalls the PE; outputs
  are downcast to bf16 on the PSUM->SBUF copy, halving output DMA; the last
  group is split 384+128 cols to shorten the post-stream drain chain.
  24 dummy FD-256 DR MMs (~5.2us busy) open the PE HAM clock gate while the
  DMA prefix lands (shorter warmups leave idle holes that re-close the gate
  at half clock, which costs far more than the warmup overrun).
- Host: gather bf16, upcast, apply scale product + bias, reshape.
"""

import numpy as np
import ml_dtypes

P = 128
B, S, D, F = 4, 2048, 2048, 8192
M = B * S  # 8192 rows of inp_mat
GRID_M, GRID_N = 4, 2
M_LOC = M // GRID_M  # 2048
N_LOC = F // GRID_N  # 4096
N_CORES = 8
N_WARM = 24  # leading dummy FD-256 DR MMs (HAM clock-gate opener, ~5.2us busy)

_PROGRAM = None  # (nc, kxm_name, kxn_name, mxn_name)


def _build_program():
    """Resident-input fp8 DoubleRow matmul: per core
    out[M_LOC, N_LOC] = kxm.T @ kxn, inputs SBUF-resident (12 MB),
    1024 DoubleRow MMs streaming at the 512-cycle floor, bf16 output."""
    global _PROGRAM
    if _PROGRAM is not None:
        return _PROGRAM
    import concourse.bacc as bacc
    import concourse.mybir as mybir
    import concourse.tile as tile

    KO = D // P  # 16 k subtiles
    KP = KO // 2  # 8 k pairs
    MT = M_LOC // P  # 16 m tiles
    NCH = N_LOC // 512  # 8 n chunks

    nc = bacc.Bacc("TRN2", target_bir_lowering=False, debug=False)
    dt = mybir.dt
    DR = mybir.MatmulPerfMode.DoubleRow

    with tile.TileContext(nc) as tc:
        # Chunk-major DRAM images: the SBUF-resident layout and the DRAM
        # layout are byte-identical, so every input DMA is a fully
        # contiguous copy (no small strided runs throttling the ramp).
        kxm = nc.dram_tensor(
            "kxm", (P, MT, KO, 128), dt.float8e4, kind="ExternalInput"
        ).ap()
        kxn = nc.dram_tensor(
            "kxn", (P, NCH, KO, 512), dt.float8e4, kind="ExternalInput"
        ).ap()
        mxn = nc.dram_tensor(
            "mxn", (P, MT, N_LOC), dt.bfloat16, kind="ExternalOutput"
        ).ap()

        with (
            tc.tile_pool(name="resident", bufs=1) as res_pool,
            tc.tile_pool(name="psum", bufs=8, space="PSUM") as psum_pool,
            tc.tile_pool(name="outp", bufs=16) as out_pool,
            tc.tile_pool(name="warm", bufs=1) as warm_pool,
        ):
            # Warm tile: stationary and moving operands of the dummy MMs both
            # read from it; one small vector memset initializes it.
            wz = warm_pool.tile([P, 2, 256], dt.float8e4, name="warm_z")
            nc.vector.memset(wz[:], 0.0)
            wps = psum_pool.tile([P, 512], dt.float32, name="ps")

            def warm_mm():
                nc.tensor.matmul(
                    wps[:, 0:256],
                    wz[:, :, 0:128],
                    wz[:],
                    start=True,
                    stop=True,
                    perf_mode=DR,
                    skip_group_check=True,
                )

            for _ in range(N_WARM):
                warm_mm()

            # Chunk-major resident tiles (t-major xw, c-major wn): each DMA
            # source range and destination range are contiguous bytes.
            xw_all = res_pool.tile([P, MT, KP, 2, 128], dt.float8e4, name="xw_all")
            wn_all = res_pool.tile([P, NCH, KP, 2, 512], dt.float8e4, name="wn_all")

            def ld_wn(jlo, jhi, c, eng=None):
                (eng or nc.sync).dma_start(
                    wn_all[:, c, jlo:jhi], kxn[:, c, 2 * jlo : 2 * jhi]
                )

            def ld_xw(jlo, jhi, t):
                nc.sync.dma_start(
                    xw_all[:, t, jlo:jhi], kxm[:, t, 2 * jlo : 2 * jhi]
                )

            def ld_wn_c(clo, chi):
                nc.sync.dma_start(wn_all[:, clo:chi], kxn[:, clo:chi])

            def ld_xw_t(tlo, thi):
                nc.sync.dma_start(xw_all[:, tlo:thi], kxm[:, tlo:thi])

            # Single sync FIFO ring, ordered to match c-major consumption:
            # group (c0,t0) unblocks pair by pair, then xw tiles land just
            # ahead of their t-groups, wn chunks well ahead of their c-phase.
            # wn j4-7 of the first chunk go out on the otherwise-idle scalar
            # HWDGE ring (issued before any output DMA), so the two rings
            # deliver the first group's gate chunks in parallel.
            ld_wn(4, 6, 0, eng=nc.scalar)
            ld_wn(6, 8, 0, eng=nc.scalar)
            ld_wn(0, 1, 0)
            ld_xw(0, 2, 0)
            ld_wn(1, 2, 0)
            ld_xw(2, 8, 0)
            ld_wn(2, 3, 0)
            ld_wn(3, 4, 0)
            ld_xw_t(1, 2)
            ld_xw_t(2, 3)
            ld_xw_t(3, 4)
            ld_xw_t(4, 6)
            ld_xw_t(6, 8)
            ld_xw_t(8, 12)
            ld_xw_t(12, MT)
            ld_wn(0, 4, 1)
            ld_wn(4, 8, 1)
            ld_wn_c(2, 4)
            ld_wn_c(4, 6)
            ld_wn_c(6, NCH)

            def group(c, t):
                ps = psum_pool.tile([P, 512], dt.float32, name="ps")
                for j in range(KP):
                    nc.tensor.matmul(
                        ps[:],
                        xw_all[:, t, j],
                        wn_all[:, c, j],
                        start=(j == 0),
                        stop=(j == KP - 1),
                        perf_mode=DR,
                    )
                oc = out_pool.tile([P, 512], dt.bfloat16, name="oc")
                nc.vector.tensor_copy(oc[:], ps[:])
                nc.scalar.dma_start(mxn[:, t, c * 512 : (c + 1) * 512], oc[:])

            def group_split(c, t):
                """Last group: two 256-col halves so the final PSUM->SBUF
                copy + output DMA chain after the last matmul is half-size
                (the first half's copy/DMA overlap the second half's MMs)."""
                for lo, w in ((0, 384), (384, 128)):
                    ps = psum_pool.tile([P, w], dt.float32, name="ps")
                    for j in range(KP):
                        nc.tensor.matmul(
                            ps[:],
                            xw_all[:, t, j],
                            wn_all[:, c, j, :, lo : lo + w],
                            start=(j == 0),
                            stop=(j == KP - 1),
                            perf_mode=DR,
                        )
                    oc = out_pool.tile([P, w], dt.bfloat16, name="oc")
                    nc.vector.tensor_copy(oc[:], ps[:])
                    nc.scalar.dma_start(
                        mxn[:, t, c * 512 + lo : c * 512 + lo + w], oc[:]
                    )

            # Main stream: c-major.
            for c in range(NCH):
                for t in range(MT):
                    if c == NCH - 1 and t == MT - 1:
                        group_split(c, t)
                    else:
                        group(c, t)

    nc.compile()
    _PROGRAM = (nc, "kxm", "kxn", "mxn")
    return _PROGRAM


def _qdq_e4m3(x, scale):
    """fp32 -> e4m3 with the reference's scale/clip semantics; TRN-fp8 view."""
    if scale != 1.0:
        x = x / np.float32(scale)
    q = np.clip(x, -448.0, 448.0).astype(ml_dtypes.float8_e4m3fn)
    return q.view(ml_dtypes.float8_e4m3)


def _make_shards(xq, wq):
    """Per-core chunk-major DRAM input images for the 4x2 grid (shared where
    slices coincide). kxm [P, MT, KO, 128] with [p,t,ko,m] = x[t*128+m, ko*128+p];
    kxn [P, NCH, KO, 512] with [p,c,ko,n] = w[ko*128+p, c*512+n] — byte layouts
    identical to the SBUF resident tiles, so input DMAs are contiguous."""
    KO, MT, NCH = D // P, M_LOC // P, N_LOC // 512
    kxm_shards = []
    for mi in range(GRID_M):
        xs = xq[mi * M_LOC : (mi + 1) * M_LOC, :]  # [M_LOC, D]
        kxm_shards.append(
            np.ascontiguousarray(
                xs.reshape(MT, P, KO, P).transpose(3, 0, 2, 1)
            )
        )
    kxn_shards = []
    for ni in range(GRID_N):
        ws = wq[:, ni * N_LOC : (ni + 1) * N_LOC]  # [D, N_LOC]
        kxn_shards.append(
            np.ascontiguousarray(
                ws.reshape(KO, P, NCH, 512).transpose(1, 2, 0, 3)
            )
        )
    return kxm_shards, kxn_shards


def _ensure_axon_hooks_stub():
    """The trimmed image lacks antenv.axon_hooks; if BASS_TRACE is set in the
    environment, run_bass_kernel_spmd would crash importing it. Provide a
    no-op hook module (bass_utils degrades gracefully on a None hook)."""
    import sys
    import types

    try:
        import antenv.axon_hooks  # noqa: F401
    except ImportError:
        mod = types.ModuleType("antenv.axon_hooks")
        mod.get_axon_ntff_profile_hook = lambda: None
        mod.set_axon_ntff_profile_hook = lambda hook: None
        sys.modules["antenv.axon_hooks"] = mod


def kernel(
    inputs,
    kernel,
    bias,
    input_scale,
    kernel_scale,
    output_grad_scale,
    input_amax_history,
    kernel_amax_history,
    output_grad_amax_history,
):
    _ensure_axon_hooks_stub()
    from concourse.bass_utils import run_bass_kernel_spmd

    nc, kxm_name, kxn_name, mxn_name = _build_program()

    x = np.asarray(inputs, dtype=np.float32).reshape(M, D)
    w = np.asarray(kernel, dtype=np.float32)
    s_in = float(np.asarray(input_scale).reshape(-1)[0])
    s_k = float(np.asarray(kernel_scale).reshape(-1)[0])

    xq = _qdq_e4m3(x, s_in)  # [M, D] fp8
    wq = _qdq_e4m3(w, s_k)  # [D, F] fp8

    kxm_shards, kxn_shards = _make_shards(xq, wq)

    in_maps = []
    for c in range(N_CORES):
        mi, ni = divmod(c, GRID_N)
        in_maps.append({kxm_name: kxm_shards[mi], kxn_name: kxn_shards[ni]})

    res = run_bass_kernel_spmd(nc, in_maps, core_ids=list(range(N_CORES)))

    out = np.empty((M, F), dtype=np.float32)
    for c in range(N_CORES):
        mi, ni = divmod(c, GRID_N)
        block = res.results[c][mxn_name]  # [P, M_LOC//P, N_LOC] bf16
        out[mi * M_LOC : (mi + 1) * M_LOC, ni * N_LOC : (ni + 1) * N_LOC] = (
            np.asarray(block)
            .view(ml_dtypes.bfloat16)
            .astype(np.float32)
            .transpose(1, 0, 2)
            .reshape(M_LOC, N_LOC)
        )

    sprod = s_in * s_k
    if sprod != 1.0:
        out *= np.float32(sprod)

    b = np.asarray(bias, dtype=np.float32)
    b = b.astype(ml_dtypes.bfloat16).astype(np.float32)
    if np.any(b):
        out += b[None, :]

    return out.reshape(B, S, F)


# revision 42
# speedup vs baseline: 1.0236x; 1.0062x over previous
"""Trainium2 kernel for nn_DenseGeneral fp8-qdq forward.

Reference computes: out = qdq_e4m3(inputs) @ qdq_e4m3(kernel) + bf16_round(bias)
(forward pass of fp8-aware DenseGeneral; scale/amax updates only live in the
custom_vjp residuals and do not affect the forward output).

Strategy:
- Host: quantize inputs/kernel to e4m3 exactly as the reference does (scales are
  ones in this problem, but general scales are folded back into the output).
  OCP e4m3fn bit patterns == TRN FP8_EXP4 for |v| <= 240, which holds here.
- 4x2 grid: shard rows of inp_mat 4-way and kernel columns 2-way; per core the
  resident inputs are xw 4 MB + wn 8 MB fp8 (12 MB SBUF), 1024 DoubleRow MMs.
- Device: all-SBUF-resident fp8 DoubleRow matmuls at the 1 col/cycle floor
  (512 cycles + ~3ns NX per MM, measured 216 ns/MM at 2.4 GHz; SwInterleave
  benches identical, so this IS the fp8 peak). All inputs stream on the sync
  HWDGE queue in c-major consumption order, with the first group's wn/xw
  split per k-pair so its j-ascending MMs unblock as each chunk lands; the
  DRAM images are chunk-major (byte-identical to the SBUF resident tiles),
  so every input DMA is a fully contiguous copy and the ramp runs ~1.5us
  faster than the strided layout.
  8 PSUM bufs (full PSUM) so copyback latency never stalls the PE; outputs
  are downcast to bf16 on the PSUM->SBUF copy, halving output DMA; the last
  group is split 384+128 cols to shorten the post-stream drain chain.
  24 dummy FD-256 DR MMs (~5.2us busy) open the PE HAM clock gate while the
  DMA prefix lands (shorter warmups leave idle holes that re-close the gate
  at half clock, which costs far more than the warmup overrun).
- Host: gather bf16, upcast, apply scale product + bias, reshape.
"""

import numpy as np
import ml_dtypes

P = 128
B, S, D, F = 4, 2048, 2048, 8192
M = B * S  # 8192 rows of inp_mat
GRID_M, GRID_N = 4, 2
M_LOC = M // GRID_M  # 2048
N_LOC = F // GRID_N  # 4096
N_CORES = 8
N_WARM = 24  # leading dummy FD-256 DR MMs (HAM clock-gate opener, ~5.2us busy)

_PROGRAM = None  # (nc, kxm_name, kxn_name, mxn_name)


def _build_program():
    """Resident-input fp8 DoubleRow matmul: per core
    out[M_LOC, N_LOC] = kxm.T @ kxn, inputs SBUF-resident (12 MB),
    1024 DoubleRow MMs streaming at the 512-cycle floor, bf16 output."""
    global _PROGRAM
    if _PROGRAM is not None:
        return _PROGRAM
    import concourse.bacc as bacc
    import concourse.mybir as mybir
    import concourse.tile as tile

    KO = D // P  # 16 k subtiles
    KP = KO // 2  # 8 k pairs
    MT = M_LOC // P  # 16 m tiles
    NCH = N_LOC // 512  # 8 n chunks

    nc = bacc.Bacc("TRN2", target_bir_lowering=False, debug=False)
    dt = mybir.dt
    DR = mybir.MatmulPerfMode.DoubleRow

    with tile.TileContext(nc) as tc:
        # Chunk-major DRAM images: the SBUF-resident layout and the DRAM
        # layout are byte-identical, so every input DMA is a fully
        # contiguous copy (no small strided runs throttling the ramp).
        kxm = nc.dram_tensor(
            "kxm", (P, MT, KO, 128), dt.float8e4, kind="ExternalInput"
        ).ap()
        kxn = nc.dram_tensor(
            "kxn", (P, NCH, KO, 512), dt.float8e4, kind="ExternalInput"
        ).ap()
        mxn = nc.dram_tensor(
            "mxn", (P, MT, N_LOC), dt.bfloat16, kind="ExternalOutput"
        ).ap()

        with (
            tc.tile_pool(name="resident", bufs=1) as res_pool,
            tc.tile_pool(name="psum", bufs=8, space="PSUM") as psum_pool,
            tc.tile_pool(name="outp", bufs=16) as out_pool,
            tc.tile_pool(name="warm", bufs=1) as warm_pool,
        ):
            # Warm tile: stationary and moving operands of the dummy MMs both
            # read from it; one small vector memset initializes it.
            wz = warm_pool.tile([P, 2, 256], dt.float8e4, name="warm_z")
            nc.vector.memset(wz[:], 0.0)
            wps = psum_pool.tile([P, 512], dt.float32, name="ps")

            def warm_mm():
                nc.tensor.matmul(
                    wps[:, 0:256],
                    wz[:, :, 0:128],
                    wz[:],
                    start=True,
                    stop=True,
                    perf_mode=DR,
                    skip_group_check=True,
                )

            for _ in range(N_WARM):
                warm_mm()

            # Chunk-major resident tiles (t-major xw, c-major wn): each DMA
            # source range and destination range are contiguous bytes.
            xw_all = res_pool.tile([P, MT, KP, 2, 128], dt.float8e4, name="xw_all")
            wn_all = res_pool.tile([P, NCH, KP, 2, 512], dt.float8e4, name="wn_all")

            def ld_wn(jlo, jhi, c, eng=None):
                (eng or nc.sync).dma_start(
                    wn_all[:, c, jlo:jhi], kxn[:, c, 2 * jlo : 2 * jhi]
                )

            def ld_xw(jlo, jhi, t):
                nc.sync.dma_start(
                    xw_all[:, t, jlo:jhi], kxm[:, t, 2 * jlo : 2 * jhi]
                )

            def ld_wn_c(clo, chi):
                nc.sync.dma_start(wn_all[:, clo:chi], kxn[:, clo:chi])

            def ld_xw_t(tlo, thi):
                nc.sync.dma_start(xw_all[:, tlo:thi], kxm[:, tlo:thi])

            # Single sync FIFO ring, ordered to match c-major consumption:
            # group (c0,t0) unblocks pair by pair, then xw tiles land just
            # ahead of their t-groups, wn chunks well ahead of their c-phase.
            # wn j2-7 of the first chunk go out on the otherwise-idle scalar
            # HWDGE ring (issued before any output DMA): the two rings carry
            # ~0.75MB each and deliver the first group's gates in parallel.
            ld_wn(2, 4, 0, eng=nc.scalar)
            ld_wn(4, 6, 0, eng=nc.scalar)
            ld_wn(6, 8, 0, eng=nc.scalar)
            ld_wn(0, 1, 0)
            ld_xw(0, 2, 0)
            ld_wn(1, 2, 0)
            ld_xw(2, 8, 0)
            ld_xw_t(1, 2)
            ld_xw_t(2, 3)
            ld_xw_t(3, 4)
            ld_xw_t(4, 6)
            ld_xw_t(6, 8)
            ld_xw_t(8, 12)
            ld_xw_t(12, MT)
            ld_wn(0, 4, 1)
            ld_wn(4, 8, 1)
            ld_wn_c(2, 4)
            ld_wn_c(4, 6)
            ld_wn_c(6, NCH)

            def group(c, t):
                ps = psum_pool.tile([P, 512], dt.float32, name="ps")
                for j in range(KP):
                    nc.tensor.matmul(
                        ps[:],
                        xw_all[:, t, j],
                        wn_all[:, c, j],
                        start=(j == 0),
                        stop=(j == KP - 1),
                        perf_mode=DR,
                    )
                oc = out_pool.tile([P, 512], dt.bfloat16, name="oc")
                nc.vector.tensor_copy(oc[:], ps[:])
                nc.scalar.dma_start(mxn[:, t, c * 512 : (c + 1) * 512], oc[:])

            def group_split(c, t):
                """Last group: two 256-col halves so the final PSUM->SBUF
                copy + output DMA chain after the last matmul is half-size
                (the first half's copy/DMA overlap the second half's MMs)."""
                for lo, w in ((0, 384), (384, 128)):
                    ps = psum_pool.tile([P, w], dt.float32, name="ps")
                    for j in range(KP):
                        nc.tensor.matmul(
                            ps[:],
                            xw_all[:, t, j],
                            wn_all[:, c, j, :, lo : lo + w],
                            start=(j == 0),
                            stop=(j == KP - 1),
                            perf_mode=DR,
                        )
                    oc = out_pool.tile([P, w], dt.bfloat16, name="oc")
                    nc.vector.tensor_copy(oc[:], ps[:])
                    # Final 128-col half goes out on the (idle) sync ring so
                    # its issue is not serialized behind the 384-col half's.
                    (nc.sync if lo else nc.scalar).dma_start(
                        mxn[:, t, c * 512 + lo : c * 512 + lo + w], oc[:]
                    )

            # Main stream: c-major.
            for c in range(NCH):
                for t in range(MT):
                    if c == NCH - 1 and t == MT - 1:
                        group_split(c, t)
                    else:
                        group(c, t)

    nc.compile()
    _PROGRAM = (nc, "kxm", "kxn", "mxn")
    return _PROGRAM


def _qdq_e4m3(x, scale):
    """fp32 -> e4m3 with the reference's scale/clip semantics; TRN-fp8 view."""
    if scale != 1.0:
        x = x / np.float32(scale)
    q = np.clip(x, -448.0, 448.0).astype(ml_dtypes.float8_e4m3fn)
    return q.view(ml_dtypes.float8_e4m3)


def _make_shards(xq, wq):
    """Per-core chunk-major DRAM input images for the 4x2 grid (shared where
    slices coincide). kxm [P, MT, KO, 128] with [p,t,ko,m] = x[t*128+m, ko*128+p];
    kxn [P, NCH, KO, 512] with [p,c,ko,n] = w[ko*128+p, c*512+n] — byte layouts
    identical to the SBUF resident tiles, so input DMAs are contiguous."""
    KO, MT, NCH = D // P, M_LOC // P, N_LOC // 512
    kxm_shards = []
    for mi in range(GRID_M):
        xs = xq[mi * M_LOC : (mi + 1) * M_LOC, :]  # [M_LOC, D]
        kxm_shards.append(
            np.ascontiguousarray(
                xs.reshape(MT, P, KO, P).transpose(3, 0, 2, 1)
            )
        )
    kxn_shards = []
    for ni in range(GRID_N):
        ws = wq[:, ni * N_LOC : (ni + 1) * N_LOC]  # [D, N_LOC]
        kxn_shards.append(
            np.ascontiguousarray(
                ws.reshape(KO, P, NCH, 512).transpose(1, 2, 0, 3)
            )
        )
    return kxm_shards, kxn_shards


def _ensure_axon_hooks_stub():
    """The trimmed image lacks antenv.axon_hooks; if BASS_TRACE is set in the
    environment, run_bass_kernel_spmd would crash importing it. Provide a
    no-op hook module (bass_utils degrades gracefully on a None hook)."""
    import sys
    import types

    try:
        import antenv.axon_hooks  # noqa: F401
    except ImportError:
        mod = types.ModuleType("antenv.axon_hooks")
        mod.get_axon_ntff_profile_hook = lambda: None
        mod.set_axon_ntff_profile_hook = lambda hook: None
        sys.modules["antenv.axon_hooks"] = mod


def kernel(
    inputs,
    kernel,
    bias,
    input_scale,
    kernel_scale,
    output_grad_scale,
    input_amax_history,
    kernel_amax_history,
    output_grad_amax_history,
):
    _ensure_axon_hooks_stub()
    from concourse.bass_utils import run_bass_kernel_spmd

    nc, kxm_name, kxn_name, mxn_name = _build_program()

    x = np.asarray(inputs, dtype=np.float32).reshape(M, D)
    w = np.asarray(kernel, dtype=np.float32)
    s_in = float(np.asarray(input_scale).reshape(-1)[0])
    s_k = float(np.asarray(kernel_scale).reshape(-1)[0])

    xq = _qdq_e4m3(x, s_in)  # [M, D] fp8
    wq = _qdq_e4m3(w, s_k)  # [D, F] fp8

    kxm_shards, kxn_shards = _make_shards(xq, wq)

    in_maps = []
    for c in range(N_CORES):
        mi, ni = divmod(c, GRID_N)
        in_maps.append({kxm_name: kxm_shards[mi], kxn_name: kxn_shards[ni]})

    res = run_bass_kernel_spmd(nc, in_maps, core_ids=list(range(N_CORES)))

    out = np.empty((M, F), dtype=np.float32)
    for c in range(N_CORES):
        mi, ni = divmod(c, GRID_N)
        block = res.results[c][mxn_name]  # [P, M_LOC//P, N_LOC] bf16
        out[mi * M_LOC : (mi + 1) * M_LOC, ni * N_LOC : (ni + 1) * N_LOC] = (
            np.asarray(block)
            .view(ml_dtypes.bfloat16)
            .astype(np.float32)
            .transpose(1, 0, 2)
            .reshape(M_LOC, N_LOC)
        )

    sprod = s_in * s_k
    if sprod != 1.0:
        out *= np.float32(sprod)

    b = np.asarray(bias, dtype=np.float32)
    b = b.astype(ml_dtypes.bfloat16).astype(np.float32)
    if np.any(b):
        out += b[None, :]

    return out.reshape(B, S, F)
